# revision 39
# baseline (speedup 1.0000x reference)
"""GCN (2-layer, PyG-style gcn_norm) Bass/Tile kernel for Trainium2, 8 NeuronCores.

Strategy (dst-partitioned message passing, dense-packed gathers, separable norm):
  - Nodes are partitioned across 8 cores by destination; every edge is routed
    to the core that owns its destination node. Self-loop + symmetric
    D^-1/2 A D^-1/2 normalization is computed on the host (index/routing
    preprocessing only).
  - The norm dinv[src]*dinv[dst] is SEPARABLE: dinv[src] is folded into the
    gather tables (x is pre-scaled on the host), and dinv[dst] COMMUTES PAST
    ReLU (dinv>0): layer 1 accumulates RAW sums, the bias enters as a rank-1
    b1 (x) sqrt(deg) matmul inside the W1 PSUM group, and a single
    dinv^2-per-node scale lands at the very end of the layer-1 epilogue
    (one factor completes layer-1's norm, one pre-pays layer-2's src factor).
  - Selection matrices are PURE 0/1 one-hots built in BATCHES: one DVE
    scalar_tensor_tensor per dst-tile builds all of that tile's S matrices
    at once -- out[p, j, w] = (iota_tiled[p, j, w] == dl[p, j]) with dl
    broadcast along w via a stride-0 AP. Sentinel dl = 300 -> all-zero row.
  - Layer 1 accumulates TRANSPOSED: p1T[feat, node] += mt^T @ S
    (lhsT = message tile, rhs = one-hot), so the epilogue needs no transpose
    before W1 and no DVE at all: ScalarE does the PSUM->SBUF copies and ReLU;
    one PE transpose puts the final [node, 64] orientation for the store.
  - The SWDGE gather drain is byte-bound, so each layer picks the smallest
    legal descriptor: layer 1 gathers single 256B rows of dinv*x
    ([100000, 128] bf16, 4 int16-reach buckets); layer 2 gathers 256B PAIR
    rows of dinv^2*(h@W2) ([50176, 128] bf16, 2 buckets) with the parity
    selecting the half (dl' = dst_local + 128*parity, two matmuls per block
    on the S_cat halves).
  - Messages are packed DENSELY: per (bucket, dst-tile) group sized to
    roundup16(max-over-cores count) so all 8 cores run one program; 128-slot
    blocks cut across group boundaries, and a block shared by adjacent tiles
    is matmul'd once per tile with the other tile's slots sentinel-masked.
  - dma_gather descriptor generation runs on one Q7 core-pair per SWDGE
    queue (~8.5ns/descriptor per pair); calls round-robin the 4 queues and
    prefetch ahead (GCN_PF=4) so multiple pairs generate descriptors
    concurrently -- the whole kernel is bound by this stream (~2.6x pair
    overlap is the NX in-flight cap), so every other engine hides under it.
  - The AllGather is SPLIT in two (hbA/hbB -> hfA/hfB, and the layer-2 pair
    table is region-remapped so bucket b == gather-half b): AG(A) runs while
    layer-1's second-half tiles still compute, and layer 2 runs TWO PASSES --
    pass 1 folds bucket-0 columns into an SBUF staging strip while AG(B) is
    in flight, pass 2 re-injects the partial and adds self/bucket-1 terms.
    No barriers: Tile's DRAM dependency tracking orders everything.
  - Self-loops never enter the gather stream: each tile's own rows arrive by
    cheap sequential HWDGE DMA and are scattered with constant selection
    matrices.
  - Layer-1 epilogue folds W2 in BEFORE the halo exchange (the AllGather
    ships h@W2, 64 wide -- half the bytes).
  - Layer-2 bias enters as a K=1 rank-1 matmul (b2 x sqrt(deg)) inside the
    PSUM accumulation; the [node, feat] output orientation is already
    row-major, so layer 2 finishes with just ReLU(scale=dinv), store.
"""

import os
from dataclasses import dataclass

import numpy as np

P = 128
PAD_DL = 300.0  # sentinel: one-hot of 300 over iota 0..255 is all-zero


@dataclass(frozen=True)
class Geom:
    n_nodes: int
    n_cores: int
    in_dim: int
    h1: int
    h2: int
    gcols1: int  # layer-1 gather-group size, in 128-slot blocks per call
    gcols2: int  # layer-2 gather-group size
    selfk: int  # tiles per sequential self-row DMA chunk
    kb: int  # S-build batch size (matmul columns per DVE op)
    mm_bf16: bool  # bf16 tables/matmul operands (accumulation stays f32)
    has_bias: bool = False  # rank-1 (b x sqrt(deg)) terms; off when b1=b2=0

    @property
    def shard(self) -> int:
        return -(-self.n_nodes // self.n_cores)

    @property
    def tiles(self) -> int:
        return -(-self.shard // P)

    @property
    def shard_pad(self) -> int:
        return self.tiles * P


def _pack_layer(core, t_idx, dl_vals, row, nbuck, bsz, tiles, n_cores, gcols):
    """Dense pad-16 packing of one layer's messages.

    Returns dict with per-core idx [P, nb*8] (wrap16), dl [P, n_mm] (f32,
    PAD_DL sentinels), plus layout: nb (blocks), calls, sched (per tile:
    list of (block, mm_col)).
    """
    buck = row // bsz
    val = (row - buck * bsz).astype(np.int16)
    assert int(row.max()) - int(buck.max()) * bsz < 32768

    gkey = (core * nbuck + buck) * tiles + t_idx
    ngrp = n_cores * nbuck * tiles
    cnt = np.bincount(gkey, minlength=ngrp).reshape(n_cores, nbuck, tiles)
    size_bt = ((cnt.max(axis=0) + 15) // 16) * 16  # [nbuck, tiles]

    # group slot starts, bucket-major; bucket totals padded to whole blocks
    start_bt = np.zeros((nbuck, tiles), dtype=np.int64)
    off = 0
    bucket_span = []  # (block0, nblocks) per bucket
    for b in range(nbuck):
        blk0 = off // P
        for t in range(tiles):
            start_bt[b, t] = off
            off += int(size_bt[b, t])
        off = ((off + P - 1) // P) * P
        bucket_span.append((blk0, off // P - blk0))
    nb = off // P

    # matmul schedule: per (b, t) the overlapped blocks, t-major mm columns
    sched = [[] for _ in range(tiles)]
    k0_bt = np.zeros((nbuck, tiles), dtype=np.int64)
    jstart_bt = np.zeros((nbuck, tiles), dtype=np.int64)
    j = 0
    for t in range(tiles):
        for b in range(nbuck):
            s, e = int(start_bt[b, t]), int(start_bt[b, t] + size_bt[b, t])
            if e == s:
                continue
            ks = range(s // P, (e + P - 1) // P)
            k0_bt[b, t] = s // P
            jstart_bt[b, t] = j
            for k in ks:
                sched[t].append((k, j))
                j += 1
    n_mm = j

    # place each message: global slot, block, lane, mm column
    order = np.argsort(gkey, kind="stable")
    gs = np.zeros(ngrp + 1, dtype=np.int64)
    np.cumsum(np.bincount(gkey, minlength=ngrp), out=gs[1:])
    pos = np.arange(len(gkey), dtype=np.int64) - gs[gkey[order]]

    ci = core[order]
    b_o, t_o = buck[order], t_idx[order]
    slot = start_bt[b_o, t_o] + pos
    kblk = slot // P
    lane = slot % P
    jcol = jstart_bt[b_o, t_o] + (kblk - k0_bt[b_o, t_o])

    idxm = np.zeros((n_cores, P, nb), dtype=np.int16)
    dlm = np.full((n_cores, P, n_mm), PAD_DL, dtype=np.float32)
    idxm[ci, lane, kblk] = val[order]
    dlm[ci, lane, jcol] = dl_vals[order]

    calls = []
    for blk0, nblk in bucket_span:
        c0 = blk0
        while c0 < blk0 + nblk:
            k = min(gcols, blk0 + nblk - c0)
            calls.append((c0, k, len(calls)))
            c0 += k
    # rewrite third field as bucket id for table slicing
    calls = [
        (c0, k, next(b for b, (b0, nn) in enumerate(bucket_span) if b0 <= c0 < b0 + nn))
        for (c0, k, _x) in calls
    ]

    def wrap16(mat):
        out = np.zeros((P, nb * 8), dtype=np.int16)
        for c0, k, _b in calls:
            seg = mat[:, c0 : c0 + k].T.reshape(-1)
            out[:16, c0 * 8 : (c0 + k) * 8] = seg.reshape(k * 8, 16).T
        return np.tile(out[:16], (8, 1))

    per_core = [dict(idx=wrap16(idxm[i]), dl=dlm[i]) for i in range(n_cores)]
    layout = dict(nb=nb, n_mm=n_mm, calls=calls, sched=sched)
    return per_core, layout


def preprocess(edge_index: np.ndarray, g: Geom):
    n, c, shard, tiles = g.n_nodes, g.n_cores, g.shard, g.tiles
    src = edge_index[0].astype(np.int64)
    dst = edge_index[1].astype(np.int64)

    deg = np.bincount(dst, minlength=n).astype(np.float32) + 1.0  # + self loop
    dinv = (1.0 / np.sqrt(deg)).astype(np.float32)

    core = dst // shard
    local = dst - core * shard
    t_idx = local // P
    dl = (local % P).astype(np.float32)

    # layer 1: single rows of x, 4 buckets
    pc1, lay1 = _pack_layer(
        core, t_idx, dl, src, 4, -(-n // 4), tiles, c, g.gcols1
    )
    # layer 2: pair rows of h@W2, 2 buckets; parity in dl'.
    # The pair table is region-remapped so bucket b == AllGather-half b:
    # region A = every core's local pairs [0, Q), region B = [Q, 2Q),
    # each region laid out rank-major (the concat order AllGather produces).
    npair2 = c * g.shard_pad // 2
    Q = g.shard_pad // 4  # pairs per core per half
    cs = src // shard
    ls = src - cs * shard
    q = ls // 2
    row2 = np.where(q < Q, cs * Q + q, c * Q + cs * Q + (q - Q))
    dlp = dl + P * (ls % 2).astype(np.float32)
    pc2, lay2 = _pack_layer(
        core, t_idx, dlp, row2, 2, -(-npair2 // 2), tiles, c, g.gcols2
    )

    dpad = np.zeros(c * g.shard_pad, dtype=np.float32)
    spad = np.ones(c * g.shard_pad, dtype=np.float32)
    for i in range(c):
        lo, hi = i * shard, (i + 1) * shard
        dpad[i * g.shard_pad : i * g.shard_pad + shard] = dinv[lo:hi]
        spad[i * g.shard_pad : i * g.shard_pad + shard] = np.sqrt(deg[lo:hi])

    per_core = [
        dict(
            gidx1=pc1[i]["idx"],
            gidx2=pc2[i]["idx"],
            dl1=pc1[i]["dl"],
            dl2=pc2[i]["dl"],
            ddst=dpad[i * g.shard_pad : (i + 1) * g.shard_pad]
            .reshape(tiles, P)
            .T.copy(),
            dd2=(dpad[i * g.shard_pad : (i + 1) * g.shard_pad] ** 2)
            .reshape(tiles, P)
            .T.copy(),
            sdeg=spad[i * g.shard_pad : (i + 1) * g.shard_pad][None, :].copy(),
        )
        for i in range(c)
    ]
    return per_core, dict(l1=lay1, l2=lay2), dinv


def build_program(g: Geom, layout):
    import concourse.bass as bass  # noqa: F401
    import concourse.mybir as mybir
    import concourse.tile as tile
    from concourse import bacc, library_config
    from concourse.bass import broadcast_tensor_aps

    f32 = mybir.dt.float32
    i16 = mybir.dt.int16
    mm_dt = mybir.dt.bfloat16 if g.mm_bf16 else mybir.dt.float32

    shard, tiles, shard_pad = g.shard, g.tiles, g.shard_pad
    ablate = set(os.environ.get("GCN_ABLATE", "").split(","))  # timing experiments
    stage = os.environ.get("GCN_STAGE", "full")  # g | gs | gsm | full
    nq = int(os.environ.get("GCN_NQ", "4"))
    sp = os.environ.get("GCN_SP", "1") == "1"
    pf = int(os.environ.get("GCN_PF", "4"))  # gather-call prefetch horizon
    qg = int(os.environ.get("GCN_QG", "2"))  # consecutive calls per queue
    # queue pattern: "g" = grouped (call//qg)%nq; "i" = interleaved period-8
    # 0,1,0,1,2,3,2,3 (4 distinct queues in every retire window)
    qpat = os.environ.get("GCN_QPAT", "i")
    _I8 = [0, 1, 0, 1, 2, 3, 2, 3]
    kb = g.kb

    s_f32 = os.environ.get("GCN_SF32", "0") == "1"
    sb_dt = f32 if s_f32 else mm_dt
    lay1, lay2 = layout["l1"], layout["l2"]
    nb1, nb2 = lay1["nb"], lay2["nb"]
    nmm1, nmm2 = lay1["n_mm"], lay2["n_mm"]
    npair2 = g.n_cores * shard_pad // 2
    elem1 = g.in_dim  # single-row width (256B bf16)
    elem2 = 2 * g.h2  # pair-row width, 128 elements (256B bf16)
    bsz1 = -(-g.n_nodes // 4)
    bsz2 = -(-npair2 // 2)
    mt_free = g.gcols1 * elem1
    assert g.gcols2 * elem2 <= mt_free
    half = tiles // 2
    assert 2 * half == tiles
    # L2 self chunks must not cross the hbA/hbB boundary (tile `half`)
    selfk2 = max(d for d in range(1, g.selfk + 1) if half % d == 0)

    nc = bacc.Bacc(
        "TRN2",
        target_bir_lowering=False,
        debug=False,
        enable_asserts=False,
        num_devices=g.n_cores,
        num_swdge_queues=nq,
        dynamic_dma_scratch_size=int(os.environ.get("GCN_SCRATCH", "98304")),
    )

    x_d = nc.dram_tensor("x", [g.n_nodes, elem1], mm_dt, kind="ExternalInput")
    xo_d = nc.dram_tensor("xown", [shard_pad, elem1], mm_dt, kind="ExternalInput")
    gi1_d = nc.dram_tensor("gidx1", [P, nb1 * 8], i16, kind="ExternalInput")
    gi2_d = nc.dram_tensor("gidx2", [P, nb2 * 8], i16, kind="ExternalInput")
    dl1_d = nc.dram_tensor("dl1", [P, nmm1], sb_dt, kind="ExternalInput")
    dl2_d = nc.dram_tensor("dl2", [P, nmm2], sb_dt, kind="ExternalInput")
    dd_d = nc.dram_tensor("ddst", [P, tiles], f32, kind="ExternalInput")
    dd2_d = nc.dram_tensor("dd2", [P, tiles], f32, kind="ExternalInput")
    sd_d = nc.dram_tensor("sdeg", [1, tiles * P], f32, kind="ExternalInput")
    w1_d = nc.dram_tensor("w1", [g.in_dim, g.h1], mm_dt, kind="ExternalInput")
    w2_d = nc.dram_tensor("w2", [g.h1, g.h2], mm_dt, kind="ExternalInput")
    b1r_d = (nc.dram_tensor("b1row", [1, g.h1], f32, kind="ExternalInput")
             if g.has_bias else None)
    b2r_d = (nc.dram_tensor("b2row", [1, g.h2], f32, kind="ExternalInput")
             if g.has_bias else None)
    io1_d = nc.dram_tensor("iot1", [P, P * kb], sb_dt, kind="ExternalInput")
    io2_d = nc.dram_tensor("iot2", [P, 2 * P * kb], sb_dt, kind="ExternalInput")
    ss_d = nc.dram_tensor("sself", [P // 2, 2 * P], mm_dt, kind="ExternalInput")
    idm_d = nc.dram_tensor("identm", [P, P], mm_dt, kind="ExternalInput")
    out_d = nc.dram_tensor("out", [shard, g.h2], f32, kind="ExternalOutput")

    hba_d = nc.dram_tensor("h_bounce_a", [shard_pad // 2, g.h2], mm_dt, kind="Internal")
    hbb_d = nc.dram_tensor("h_bounce_b", [shard_pad // 2, g.h2], mm_dt, kind="Internal")
    hfa_d = nc.dram_tensor(
        "h_full_a", [npair2 // 2, elem2], mm_dt, kind="Internal", addr_space="Shared"
    )
    hfb_d = nc.dram_tensor(
        "h_full_b", [npair2 // 2, elem2], mm_dt, kind="Internal", addr_space="Shared"
    )

    with tile.TileContext(nc) as tc:
        with (
            tc.tile_pool(name="const", bufs=1) as cpool,
            tc.tile_pool(name="msg", bufs=int(os.environ.get("GCN_MBUFS", "21"))) as mpool,
            tc.tile_pool(name="selfp", bufs=3) as fpool,
            tc.tile_pool(name="sel", bufs=int(os.environ.get("GCN_SBUFS", "3"))) as spool,
            tc.tile_pool(name="act", bufs=4) as apool,
            tc.tile_pool(name="psum", bufs=2, space="PSUM") as ppool,
        ):
            nc.gpsimd.load_library(library_config.mlp)

            # index/dl tables first: the first gather calls need them
            gi1_s = cpool.tile([P, nb1 * 8], i16, tag="gidx1")
            nc.sync.dma_start(out=gi1_s[:], in_=gi1_d[:, :])
            dl1_s = cpool.tile([P, nmm1], sb_dt, tag="dl1")
            nc.sync.dma_start(out=dl1_s[:], in_=dl1_d[:, :])

            io1 = cpool.tile([P, P * kb], sb_dt, tag="iot1")
            nc.sync.dma_start(out=io1[:], in_=io1_d[:, :])
            io2 = cpool.tile([P, 2 * P * kb], sb_dt, tag="iot2")
            nc.sync.dma_start(out=io2[:], in_=io2_d[:, :])
            sself = cpool.tile([P // 2, 2 * P], mm_dt, tag="sself")
            nc.sync.dma_start(out=sself[:], in_=ss_d[:, :])
            ident = cpool.tile([P, P], mm_dt, tag="ident")
            nc.sync.dma_start(out=ident[:], in_=idm_d[:, :])

            w1_s = cpool.tile([g.in_dim, g.h1], mm_dt, tag="w1")
            nc.sync.dma_start(out=w1_s[:], in_=w1_d[:, :])
            w2_s = cpool.tile([g.h1, g.h2], mm_dt, tag="w2")
            nc.sync.dma_start(out=w2_s[:], in_=w2_d[:, :])
            if g.has_bias:
                b1r_s = cpool.tile([1, g.h1], f32, tag="b1r")
                nc.sync.dma_start(out=b1r_s[:], in_=b1r_d[:, :])
                b2r_s = cpool.tile([1, g.h2], f32, tag="b2r")
                nc.sync.dma_start(out=b2r_s[:], in_=b2r_d[:, :])
            dd_s = cpool.tile([P, tiles], f32, tag="ddst")
            nc.sync.dma_start(out=dd_s[:], in_=dd_d[:, :])
            dd2_s = cpool.tile([P, tiles], f32, tag="dd2")
            nc.sync.dma_start(out=dd2_s[:], in_=dd2_d[:, :])
            sd_s = cpool.tile([1, tiles * P], f32, tag="sdeg")
            nc.sync.dma_start(out=sd_s[:], in_=sd_d[:, :])

            # warm-up: one tiny gather per SWDGE queue so each Q7 pair's
            # IRAM library load + HAM ramp happens under the const DMAs.
            # idxs reinterpret iot1's bf16 bits as int16 (all positive,
            # < 32768, in-bounds rows of x); results are never read.
            if "gather" not in ablate:
                wu = mpool.tile([P, mt_free], mm_dt, tag="msg")
                for q in range(nq):
                    nc.gpsimd.dma_gather(
                        queue_num=q,
                        out_ap=wu[:].rearrange("p (k d) -> p k d", d=elem1)[:, :1, :],
                        in_ap=x_d[0:32768, :],
                        idxs_ap=io1[:, 0:8].bitcast(i16),
                        num_idxs=16,
                        num_idxs_reg=16,
                        elem_size=elem1,
                        single_packet=sp,
                    )

            # layer-2 index/dl tables preloaded too (before layer 1 runs)
            gi2_s = cpool.tile([P, nb2 * 8], i16, tag="gidx2")
            nc.sync.dma_start(out=gi2_s[:], in_=gi2_d[:, :])
            dl2_s = cpool.tile([P, nmm2], sb_dt, tag="dl2")
            nc.sync.dma_start(out=dl2_s[:], in_=dl2_d[:, :])

            def layer(gidx_s, dl_s, lay, table_bucket_ap, self_chunk_ap,
                      elem, fh, pair, transposed, epilogue, selfk,
                      stage_sb=None):
                nchunk = -(-tiles // selfk)
                calls = lay["calls"]
                sched = lay["sched"]
                col2call = np.zeros(lay["nb"], dtype=np.int64)
                for ci_, (c0, k, _b) in enumerate(calls):
                    col2call[c0 : c0 + k] = ci_
                # staged (two-pass) mode: pass 0 folds bucket-0 columns into
                # an SBUF staging strip (so its gathers overlap the second
                # AllGather half), pass 1 re-injects it and adds the rest
                if stage_sb is not None:
                    blk2buck = np.zeros(lay["nb"], dtype=np.int64)
                    for (c0, k, b) in calls:
                        blk2buck[c0 : c0 + k] = b
                    sched_p = [
                        [[kj for kj in sched[t] if blk2buck[kj[0]] == b_]
                         for t in range(tiles)]
                        for b_ in (0, 1)
                    ]
                else:
                    sched_p = [sched]
                msg_tiles: dict[int, object] = {}
                self_tiles: dict[int, object] = {}
                wid = 2 * P if pair else P
                io_s = io2 if pair else io1
                # hoist the common num_idxs register write: one MOVE per
                # layer instead of one per gather call (fewer SPMD pops)
                full_k = max(k for _c0, k, _b in calls)
                nreg = nc.gpsimd.to_reg(full_k * P)

                def ensure_call(ci_: int):
                    if ci_ >= len(calls) or ci_ in msg_tiles:
                        return
                    c0, k, b = calls[ci_]
                    mt = mpool.tile([P, mt_free], mm_dt, tag="msg")
                    if "gather" in ablate:
                        msg_tiles[ci_] = mt
                        return
                    nc.gpsimd.dma_gather(
                        queue_num=(_I8[ci_ % 8] % nq) if qpat == "i"
                        else (ci_ // qg) % nq,
                        out_ap=mt[:].rearrange("p (k d) -> p k d", d=elem)[:, :k, :],
                        in_ap=table_bucket_ap(b),
                        idxs_ap=gidx_s[:, c0 * 8 : (c0 + k) * 8],
                        num_idxs=k * P,
                        num_idxs_reg=nreg if k == full_k else k * P,
                        elem_size=elem,
                        single_packet=sp,
                    )
                    msg_tiles[ci_] = mt

                def ensure_self(ch: int):
                    if ch in self_tiles:
                        return
                    t0 = ch * selfk
                    kk = min(selfk, tiles - t0)
                    prt = P if not pair else P // 2
                    ft = fpool.tile([P, selfk * elem1], mm_dt, tag="selfmt")
                    nc.sync.dma_start(
                        out=ft[:prt, : kk * elem].rearrange(
                            "i (t e) -> i t e", e=elem
                        ),
                        in_=self_chunk_ap(t0, kk),
                    )
                    self_tiles[ch] = ft

                def build_s(sch_t):
                    # one-hot batch: S[p, j, w] = (iota[p, j, w] == dl[p, j])
                    n_t = len(sch_t)
                    if n_t == 0:
                        return [], -1
                    j0 = sch_t[0][1]
                    chunks = []
                    done = 0
                    while done < n_t:
                        kk = min(kb, n_t - done)
                        s_c = spool.tile([P, 2 * P * kb], mm_dt, tag="S")
                        io3 = io_s[:, : kk * wid].rearrange(
                            "p (k w) -> p k w", w=wid
                        )
                        dl3 = dl_s[:, j0 + done : j0 + done + kk].rearrange(
                            "p (k o) -> p k o", o=1
                        )
                        io3b, dl3b = broadcast_tensor_aps(io3, dl3)
                        out3 = s_c[:, : kk * wid].rearrange(
                            "p (k w) -> p k w", w=wid
                        )
                        nc.vector.scalar_tensor_tensor(
                            out3, io3b, 1.0, dl3b,
                            op0=mybir.AluOpType.bypass,
                            op1=mybir.AluOpType.is_equal,
                        )
                        chunks.append(s_c)
                        done += kk
                    return chunks, j0

                for pass_i, sch in enumerate(sched_p):
                    first_pass = stage_sb is not None and pass_i == 0
                    last_pass = pass_i == len(sched_p) - 1
                    for t in range(tiles):
                        if last_pass:
                            ensure_self(t // selfk)
                            if t // selfk + 1 < nchunk and t % selfk >= selfk - 2:
                                ensure_self(t // selfk + 1)
                        if stage in ("gs", "gsm", "full"):
                            s_chunks, j0 = build_s(sch[t])
                        started = False
                        if stage in ("gsm", "full") and last_pass:
                            p1 = ppool.tile([P, P], f32, tag="p1", space="PSUM")
                            ft = self_tiles[t // selfk]
                            so = (t % selfk) * elem
                            if transposed:
                                # p1T[feat, node] += ft^T @ ident
                                nc.tensor.matmul(
                                    p1[:, :],
                                    lhsT=ft[:, so : so + fh],
                                    rhs=ident[:, :],
                                    start=True,
                                    stop=not sch[t],
                                )
                            elif pair:
                                # rank-1 bias then pair-half self scatter
                                nc.tensor.matmul(
                                    p1[:, :fh],
                                    lhsT=sd_s[:, t * P : (t + 1) * P],
                                    rhs=b2r_s[:, :],
                                    start=True,
                                    stop=False,
                                )
                                nc.tensor.matmul(
                                    p1[:, :fh],
                                    lhsT=sself[:, 0:P],
                                    rhs=ft[: P // 2, so : so + fh],
                                    start=False,
                                    stop=False,
                                )
                                nc.tensor.matmul(
                                    p1[:, :fh],
                                    lhsT=sself[:, P : 2 * P],
                                    rhs=ft[: P // 2, so + fh : so + 2 * fh],
                                    start=False,
                                    stop=False,
                                )
                                if stage_sb is not None:
                                    # re-inject the staged bucket-0 partial
                                    nc.tensor.matmul(
                                        p1[:, :fh],
                                        lhsT=ident[:, :],
                                        rhs=stage_sb[:, t * fh : (t + 1) * fh],
                                        start=False,
                                        stop=not sch[t],
                                    )
                            started = True
                        elif stage in ("gsm", "full"):
                            p1 = ppool.tile([P, P], f32, tag="p2", space="PSUM")
                        for i_, (k, jmm) in enumerate(sch[t]):
                            ci_ = int(col2call[k])
                            # burst-fill the queue pipeline at pass start
                            depth = pf if (t or i_) else 2 * pf
                            for ahead in range(depth):
                                ensure_call(ci_ + ahead)
                            if stage in ("g", "gs"):
                                continue
                            off = k - calls[ci_][0]
                            mt = msg_tiles[ci_]
                            last = i_ == len(sch[t]) - 1
                            s_t = s_chunks[(jmm - j0) // kb]
                            sc = ((jmm - j0) % kb) * wid
                            if transposed:
                                # p1T[feat, node] += mt^T @ S
                                nc.tensor.matmul(
                                    p1[:, :],
                                    lhsT=mt[:, off * elem : off * elem + fh],
                                    rhs=s_t[:, sc : sc + P],
                                    start=False,
                                    stop=last,
                                )
                            else:
                                nc.tensor.matmul(
                                    p1[:, :fh],
                                    lhsT=s_t[:, sc : sc + P],
                                    rhs=mt[:, off * elem : off * elem + fh],
                                    start=not started,
                                    stop=last and not pair,
                                )
                                started = True
                                if pair:
                                    nc.tensor.matmul(
                                        p1[:, :fh],
                                        lhsT=s_t[:, sc + P : sc + 2 * P],
                                        rhs=mt[:, off * elem + fh : off * elem + 2 * fh],
                                        start=False,
                                        stop=last,
                                    )
                        if stage in ("g", "gs", "gsm"):
                            continue
                        if first_pass:
                            if sch[t]:
                                nc.scalar.activation(
                                    stage_sb[:, t * fh : (t + 1) * fh],
                                    p1[:, :fh], copy_fn,
                                )
                            else:
                                nc.vector.memset(
                                    stage_sb[:, t * fh : (t + 1) * fh], 0.0
                                )
                        else:
                            epilogue(t, p1)

            relu = mybir.ActivationFunctionType.Relu
            ident_fn = mybir.ActivationFunctionType.Identity
            copy_fn = mybir.ActivationFunctionType.Copy

            def epi_l1(t, p1):
                # p1 = p1T[feat, node] RAW aggregate (dinv[src] in table rows).
                # dinv[dst] commutes past ReLU; b1 enters as rank-1 b1 x sdeg.
                a2 = apool.tile([P, P], mm_dt, tag="a2")
                nc.scalar.activation(a2[:, :], p1[:, :], copy_fn)
                p2 = ppool.tile([P, P], f32, tag="p2", space="PSUM")
                nc.tensor.matmul(
                    p2[:, :], lhsT=b1r_s[:, :], rhs=sd_s[:, t * P : (t + 1) * P],
                    start=True, stop=False,
                )
                nc.tensor.matmul(
                    p2[:, :], lhsT=w1_s[:, :], rhs=a2[:, :], start=False, stop=True
                )
                ht = apool.tile([P, P], mm_dt, tag="ht")
                nc.scalar.activation(ht[:, :], p2[:, :], relu)
                p2b = ppool.tile([P, P], f32, tag="p2b", space="PSUM")
                nc.tensor.matmul(
                    p2b[: g.h2, :], lhsT=w2_s[:, : g.h2], rhs=ht[:, :],
                    start=True, stop=True,
                )
                h2t = apool.tile([P, P], mm_dt, tag="h2t")
                nc.scalar.activation(h2t[: g.h2, :], p2b[: g.h2, :], copy_fn)
                pt = ppool.tile([P, P], mm_dt, tag="ptr", space="PSUM")
                nc.tensor.transpose(pt[:, : g.h2], h2t[: g.h2, :], ident[: g.h2, : g.h2])
                hrow = apool.tile([P, P], mm_dt, tag="hrow")
                nc.scalar.activation(
                    hrow[:, : g.h2], pt[:, : g.h2], ident_fn,
                    scale=dd2_s[:, t : t + 1],
                )
                hb, tt = (hba_d, t) if t < half else (hbb_d, t - half)
                nc.sync.dma_start(out=hb[tt * P : (tt + 1) * P, :], in_=hrow[:, : g.h2])

            def epi_l2(t, p1):
                # p1[node, h2] aggregated (incl. rank-1 bias term);
                # out = relu(dinv[dst] * p1) -- already row-major
                hrow = apool.tile([P, P], f32, tag="hrowf")
                nc.scalar.activation(
                    hrow[:, : g.h2], p1[:, : g.h2], relu,
                    scale=dd_s[:, t : t + 1],
                )
                rows = min(P, shard - t * P)
                nc.sync.dma_start(
                    out=out_d[t * P : t * P + rows, :], in_=hrow[:rows, : g.h2]
                )

            def tab1(b):
                lo = b * bsz1
                hi = min(g.n_nodes, lo + bsz1)
                return x_d[lo:hi, :]

            def tab2(b):
                return (hfa_d if b == 0 else hfb_d)[:, :]

            def self1(t0, kk):
                return xo_d[t0 * P : (t0 + kk) * P, :].rearrange(
                    "(t i) e -> i t e", i=P
                )

            def self2(t0, kk):
                hb, tt = (hba_d, t0) if t0 < half else (hbb_d, t0 - half)
                return hb[tt * P : (tt + kk) * P, :].rearrange(
                    "(t i b) c -> i t (b c)", i=64, b=2
                )

            layer(gi1_s, dl1_s, lay1, tab1, self1, elem1, g.in_dim, False,
                  True, epi_l1, g.selfk)

            nobar = os.environ.get("GCN_NOBAR", "1") == "1"
            if not nobar:
                tc.strict_bb_all_engine_barrier()
            if os.environ.get("GCN_NOCC", "0") == "1":  # debug: skip collective
                nc.sync.dma_start(
                    out=hfa_d[: shard_pad // 4, :],
                    in_=hba_d[:, :].rearrange("(a b) c -> a (b c)", b=2),
                )
                nc.sync.dma_start(
                    out=hfb_d[: shard_pad // 4, :],
                    in_=hbb_d[:, :].rearrange("(a b) c -> a (b c)", b=2),
                )
            else:
                # bf16 AllGather was observed to wedge the exec unit at
                # >=512KB per rank; it is pure data movement, so ship the
                # same bytes as f32. Two half-gathers: AG(A) completes while
                # layer-1's B-half tiles still run, and layer-2's bucket-0
                # gathers (region A) overlap AG(B).
                for hb, hf in ((hba_d, hfa_d), (hbb_d, hfb_d)):
                    cc_in = hb.ap() if not g.mm_bf16 else hb.ap().bitcast(f32)
                    cc_out = hf.ap() if not g.mm_bf16 else hf.ap().bitcast(f32)
                    nc.gpsimd.collective_compute(
                        "AllGather",
                        mybir.AluOpType.bypass,
                        replica_groups=[list(range(g.n_cores))],
                        ins=[cc_in.opt()],
                        outs=[cc_out.opt()],
                    )
            if not nobar:
                tc.strict_bb_all_engine_barrier()

            stage_sb = cpool.tile([P, tiles * g.h2], mm_dt, tag="stage")
            layer(gi2_s, dl2_s, lay2, tab2, self2, elem2, g.h2, True,
                  False, epi_l2, selfk2, stage_sb=stage_sb)

    nc.compile()
    return nc


_PROGRAM_CACHE: dict = {}
LAST_RESULTS = None  # BassKernelResults of the most recent kernel() call


def _layout_key(layout):
    def lk(lay):
        return (
            lay["nb"],
            lay["n_mm"],
            tuple(lay["calls"]),
            tuple(tuple(s) for s in lay["sched"]),
        )

    return (lk(layout["l1"]), lk(layout["l2"]))


def _get_program(g: Geom, layout):
    key = (g, _layout_key(layout))
    if key not in _PROGRAM_CACHE:
        _PROGRAM_CACHE[key] = build_program(g, layout)
    return _PROGRAM_CACHE[key]


def host_consts(g: Geom):
    import ml_dtypes

    tdt = ml_dtypes.bfloat16 if g.mm_bf16 else np.float32
    sdt = np.float32 if os.environ.get("GCN_SF32", "0") == "1" else tdt
    iot1 = np.tile(np.arange(P, dtype=np.float32), (P, g.kb)).astype(sdt)
    iot2 = np.tile(np.arange(2 * P, dtype=np.float32), (P, g.kb)).astype(sdt)
    sself = np.zeros((P // 2, 2 * P), dtype=np.float32)
    for i in range(P // 2):
        sself[i, 2 * i] = 1.0
        sself[i, P + 2 * i + 1] = 1.0
    ident = np.eye(P, dtype=np.float32)
    return dict(iot1=iot1, iot2=iot2, sself=sself.astype(tdt),
                identm=ident.astype(tdt))


def run(x, edge_index, W1, b1, W2, b2, g: Geom, trace: bool = False):
    global LAST_RESULTS
    import ml_dtypes
    from concourse.bass_utils import run_bass_kernel_spmd

    import dataclasses
    has_bias = bool(np.any(np.asarray(b1)) or np.any(np.asarray(b2)))
    g = dataclasses.replace(g, has_bias=has_bias)
    per_core, layout, dinv = preprocess(np.asarray(edge_index), g)
    nc = _get_program(g, layout)

    tdt = ml_dtypes.bfloat16 if g.mm_bf16 else np.float32
    sdt = np.float32 if os.environ.get("GCN_SF32", "0") == "1" else tdt
    consts = host_consts(g)
    xs = np.asarray(x) * dinv[:, None]  # fold dinv[src] into the table
    x_t = np.ascontiguousarray(xs).astype(tdt)
    w1_t = np.asarray(W1).astype(tdt)
    w2_t = np.asarray(W2).astype(tdt)
    b1_t = np.asarray(b1).astype(np.float32)[None, :]
    b2_t = np.asarray(b2).astype(np.float32)[None, :]

    xo_pad = np.zeros((g.shard_pad, g.in_dim), dtype=tdt)
    in_maps = []
    for i, pc in enumerate(per_core):
        lo = i * g.shard
        xo = xo_pad.copy()
        xo[: g.shard] = x_t[lo : lo + g.shard]
        im = dict(
            x=x_t, xown=xo, gidx1=pc["gidx1"], gidx2=pc["gidx2"],
            dl1=pc["dl1"].astype(sdt), dl2=pc["dl2"].astype(sdt),
            ddst=pc["ddst"], dd2=pc["dd2"],
            w1=w1_t, w2=w2_t, **consts,
        )
        if has_bias:
            im.update(sdeg=pc["sdeg"], b1row=b1_t, b2row=b2_t)
        in_maps.append(im)

    core_ids = list(range(g.n_cores))
    if trace:
        try:
            res = run_bass_kernel_spmd(
                nc, in_maps, core_ids=core_ids, trace=True, trace_cores=[0]
            )
        except Exception as e:  # fall back to an untraced run
            print(f"[kernel] traced run failed ({type(e).__name__}: {e}); retrying untraced")
            res = run_bass_kernel_spmd(nc, in_maps, core_ids=core_ids)
    else:
        res = run_bass_kernel_spmd(nc, in_maps, core_ids=core_ids)
    LAST_RESULTS = res
    out = np.concatenate([r["out"] for r in res.results], axis=0)
    return out[: g.n_nodes]


_FULL = Geom(
    n_nodes=100000,
    n_cores=8,
    in_dim=128,
    h1=128,
    h2=64,
    gcols1=int(os.environ.get("GCN_GCOLS1", "8")),
    gcols2=int(os.environ.get("GCN_GCOLS2", "8")),
    selfk=int(os.environ.get("GCN_SELFK", "14")),
    kb=int(os.environ.get("GCN_KB", "8")),
    mm_bf16=os.environ.get("GCN_F32", "0") != "1",
)


def kernel(x, edge_index, W1, b1, W2, b2):
    trace = os.environ.get("GCN_TRACE", "0") == "1"
    return run(x, edge_index, W1, b1, W2, b2, _FULL, trace=trace)
